# revision 9
# baseline (speedup 1.0000x reference)
"""DeformConv2d (B=8, C=128, H=W=64, K=3x3, pad 1, stride 1) on 8 trn2 NeuronCores.

Data-parallel over batch: core b handles image b. Per core:
  - The padded image lives in DRAM position-major with all 4 bilinear corner
    rows packed per entry: xt4[r] = [pos r | r+1 | r+68 | r+69], each 128ch
    bf16, r = y*68+x over a 68x68 zero-ringed grid (ring width 2). One
    dma_gather index fetches all 4 corners of one sample point for all 128
    channels, transposed into channel-on-partition SBUF layout [c, l, i]
    (l = corner lane y0x0,y0x1,y1x0,y1x1).
  - Bilinear corner weights are computed on DVE in natural [128, 288] layout,
    staged to DRAM lane-major, and broadcast to all 128 partitions with a
    stride-0-source DMA (one 32KB broadcast per tap).
  - DVE multiplies gathered corners by broadcast weights (bf16); PE matmuls
    accumulate 9 taps x 4 corner lanes into per-quarter PSUM [128, 1024].
  - Tail: psum + bias -> fp32 output quarter, DMA to DRAM.
dma_gather note: transpose-mode gathers hang above ~1024 descriptors in
flight (SWDGE ring capacity); chunks are capped at 896 indices per call.
"""
import numpy as np
import ml_dtypes

B, CIN, H, W = 8, 128, 64, 64
COUT, KH, KW = 128, 3, 3
K = KH * KW
HO, WO = 64, 64
P = 128
NPOS = HO * WO              # 4096 positions
Q = NPOS // P               # 32 idx-cols in natural [128, K*Q] layout
PADR = 2
WP = W + 2 * PADR           # 68
HP = H + 2 * PADR           # 68
NE = HP * WP                # 4624 padded positions
FB = 1024.0                 # bias to keep pre-floor coords positive
NH = NPOS // 2              # 2048 positions per half
NQ = NPOS // 4              # 1024 positions per psum quarter
IDXC = K * Q * 8            # 2304 wrapped idx cols (k, q, g)
CHUNKS = [(0, 896), (896, 896), (1792, 256)]  # per-half gather chunks


def _build_kernel():
    import concourse.bacc as bacc
    import concourse.mybir as mybir
    import concourse.tile as tile
    import concourse.library_config as library_config

    nc = bacc.Bacc("TRN2", target_bir_lowering=False, debug=False,
                   num_devices=8)
    f32, bf16, i16 = mybir.dt.float32, mybir.dt.bfloat16, mybir.dt.int16
    i32 = mybir.dt.int32
    ALU = mybir.AluOpType

    xt_d = nc.dram_tensor("xt", [NE, 4 * P], bf16, kind="ExternalInput")
    off_d = nc.dram_tensor("off2", [P, 2 * K * Q], f32, kind="ExternalInput")
    tab_d = nc.dram_tensor("tab2", [P, 2 * K * Q], f32, kind="ExternalInput")
    wmat_d = nc.dram_tensor("wmat", [P, K * COUT], bf16, kind="ExternalInput")
    bias_d = nc.dram_tensor("bias", [P, 1], f32, kind="ExternalInput")
    out_d = nc.dram_tensor("out", [P, NPOS], f32, kind="ExternalOutput")

    with tile.TileContext(nc) as tc:
        with tc.tile_pool(name="const", bufs=1) as cpool, \
             tc.tile_pool(name="gen", bufs=1) as gpool, \
             tc.tile_pool(name="wbc", bufs=2) as wpool, \
             tc.tile_pool(name="gath", bufs=6) as gapool, \
             tc.tile_pool(name="mm", bufs=2) as mpool, \
             tc.tile_pool(name="outp", bufs=2) as opool, \
             tc.tile_pool(name="dramw", bufs=1, space="DRAM") as dpool, \
             tc.tile_pool(name="ps", bufs=1, space="PSUM") as pspool:

            wrow = dpool.tile([K, 4 * NPOS], mybir.dt.bfloat16)

            nc.gpsimd.load_library(library_config.mlp)

            # ---------------- stage 0: input loads --------------------------
            off2 = cpool.tile([P, 2 * K * Q], f32)
            nc.sync.dma_start(out=off2[:], in_=off_d.ap())
            tab2 = cpool.tile([P, 2 * K * Q], f32)
            nc.sync.dma_start(out=tab2[:], in_=tab_d.ap())
            wmat = cpool.tile([P, K * COUT], bf16)
            nc.scalar.dma_start(out=wmat[:], in_=wmat_d.ap())
            bias = cpool.tile([P, 1], f32)
            nc.scalar.dma_start(out=bias[:], in_=bias_d.ap())

            # ---------------- stage 1: coords, weights, indices -------------
            NG = K * Q  # 288
            py = gpool.tile([P, NG], f32)
            px = gpool.tile([P, NG], f32)
            # py = offy + (hob + FB)   (tab already holds ho - 1 + ky + FB)
            nc.vector.tensor_tensor(out=py[:], in0=off2[:, 0:NG],
                                    in1=tab2[:, 0:NG], op=ALU.add)
            nc.vector.tensor_tensor(out=px[:], in0=off2[:, NG:2 * NG],
                                    in1=tab2[:, NG:2 * NG], op=ALU.add)

            # floor robust to cast rounding mode (trunc in sim, RN on hw)
            def floor_frac(pb, sfx):
                i0 = gpool.tile([P, NG], i32, tag="ffi" + sfx)
                nc.vector.tensor_copy(out=i0[:], in_=pb[:])
                f0 = gpool.tile([P, NG], f32, tag="fff" + sfx)
                nc.vector.tensor_copy(out=f0[:], in_=i0[:])
                lr = gpool.tile([P, NG], f32, tag="ffl" + sfx)
                nc.vector.tensor_tensor(out=lr[:], in0=pb[:], in1=f0[:],
                                        op=ALU.subtract)
                adj = gpool.tile([P, NG], f32, tag="ffa" + sfx)
                nc.vector.tensor_scalar(out=adj[:], in0=lr[:], scalar1=0.0,
                                        scalar2=None, op0=ALU.is_lt)
                fr = gpool.tile([P, NG], f32, tag="ffr" + sfx)
                nc.vector.tensor_tensor(out=fr[:], in0=lr[:], in1=adj[:],
                                        op=ALU.add)
                fl = gpool.tile([P, NG], f32, tag="ffo" + sfx)
                nc.vector.tensor_tensor(out=fl[:], in0=f0[:], in1=adj[:],
                                        op=ALU.subtract)
                return fl, fr
            y0f, ly = floor_frac(py, "y")
            x0f, lx = floor_frac(px, "x")

            # clamp biased corner coords to [-PADR, 64]+FB
            ycl = gpool.tile([P, NG], f32)
            xcl = gpool.tile([P, NG], f32)
            nc.vector.tensor_scalar(out=ycl[:], in0=y0f[:], scalar1=FB - PADR,
                                    scalar2=FB + 64.0, op0=ALU.max, op1=ALU.min)
            nc.vector.tensor_scalar(out=xcl[:], in0=x0f[:], scalar1=FB - PADR,
                                    scalar2=FB + 64.0, op0=ALU.max, op1=ALU.min)
            # row idx r = (ycl-FB+2)*68 + (xcl-FB+2) = 68*ycl + xcl - 69*FB + 138
            rf = gpool.tile([P, NG], f32)
            nc.vector.scalar_tensor_tensor(
                out=rf[:], in0=ycl[:], scalar=float(WP), in1=xcl[:],
                op0=ALU.mult, op1=ALU.add)
            r16 = gpool.tile([P, NG], i16)
            nc.vector.tensor_scalar(out=r16[:], in0=rf[:],
                                    scalar1=-(WP + 1.0) * FB + 2 * WP + 2.0,
                                    scalar2=None, op0=ALU.add)

            # bilinear corner-weight products, lane-major (l = 2*jy + jx)
            omy = gpool.tile([P, NG], f32)
            omx = gpool.tile([P, NG], f32)
            nc.vector.tensor_scalar(out=omy[:], in0=ly[:], scalar1=-1.0,
                                    scalar2=1.0, op0=ALU.mult, op1=ALU.add)
            nc.vector.tensor_scalar(out=omx[:], in0=lx[:], scalar1=-1.0,
                                    scalar2=1.0, op0=ALU.mult, op1=ALU.add)
            wpre = gpool.tile([P, K * 4 * Q], bf16)
            wv = wpre[:].rearrange("p (k l q) -> p k l q", k=K, l=4, q=Q)
            omy3 = omy[:].rearrange("p (k q) -> p k q", k=K, q=Q)
            ly3 = ly[:].rearrange("p (k q) -> p k q", k=K, q=Q)
            omx3 = omx[:].rearrange("p (k q) -> p k q", k=K, q=Q)
            lx3 = lx[:].rearrange("p (k q) -> p k q", k=K, q=Q)
            nc.vector.tensor_tensor(out=wv[:, :, 0], in0=omy3, in1=omx3,
                                    op=ALU.mult)  # y0 x0
            nc.vector.tensor_tensor(out=wv[:, :, 1], in0=omy3, in1=lx3,
                                    op=ALU.mult)  # y0 x1
            nc.vector.tensor_tensor(out=wv[:, :, 2], in0=ly3, in1=omx3,
                                    op=ALU.mult)  # y1 x0
            nc.vector.tensor_tensor(out=wv[:, :, 3], in0=ly3, in1=lx3,
                                    op=ALU.mult)  # y1 x1
            # stage to DRAM: wrow[k, l*4096 + q*128 + Pp] = wpre[Pp, k, l, q]
            wrow_v = wrow[:].rearrange(
                "k (l q p) -> p k l q", l=4, q=Q, p=P)
            nc.sync.dma_start(out=wrow_v, in_=wpre[:])

            # wrapped idx table: idx16[r, k*256 + q*8 + g] = r16[g*16+r, k*Q+q]
            # (gather slot i = q*128 + g*16 + r = position p, identity order)
            # High priority: gathers gate on these; keep them ahead of the
            # big weight broadcasts in the schedule.
            idx16 = gpool.tile([P, IDXC], i16)
            with tc.high_priority():
                idxv = idx16[0:16, :].rearrange("p (k q g) -> p k q g",
                                                k=K, q=Q, g=8)
                r16v = r16[:].rearrange("(g p) (k q) -> g p k q", g=8, p=16,
                                        k=K, q=Q)
                dma_engs = [nc.sync, nc.scalar, nc.sync, nc.scalar,
                            nc.sync, nc.scalar, nc.sync, nc.scalar]
                for g in range(8):
                    dma_engs[g].dma_start(out=idxv[:, :, :, g], in_=r16v[g])
                # replicate to all 128 partitions (16 -> 32 -> 64 -> 128)
                nc.sync.dma_start(out=idx16[16:32, :], in_=idx16[0:16, :])
                nc.scalar.dma_start(out=idx16[32:64, :], in_=idx16[0:32, :])
                nc.sync.dma_start(out=idx16[64:128, :], in_=idx16[0:64, :])

            # ---------------- stage 2: per-tap gather/mult/matmul -----------
            # psum quarters: (h, q) -> positions [h*2048 + q*1024, +1024)
            ps = [[None, None], [None, None]]
            for h in range(2):
                for q in range(2):
                    psq = pspool.tile([P, NQ], f32, tag=f"ps{h}{q}",
                                      name=f"ps{h}{q}")
                    ps[h][q] = psq

            for k in range(K):
                wbc = wpool.tile([P, 4 * NPOS], bf16, tag="wb")
                # split each 4MB broadcast across both HWDGE engines
                nc.sync.dma_start(
                    out=wbc[:, 0:2 * NPOS],
                    in_=wrow[k:k + 1, 0:2 * NPOS].to_broadcast((P, 2 * NPOS)))
                nc.scalar.dma_start(
                    out=wbc[:, 2 * NPOS:4 * NPOS],
                    in_=wrow[k:k + 1, 2 * NPOS:4 * NPOS].to_broadcast(
                        (P, 2 * NPOS)))
                wbc4 = wbc[:].rearrange("p (l i) -> p l i", l=4)
                lhsT = wmat[:, k * COUT:(k + 1) * COUT]
                for h in range(2):
                    m = mpool.tile([P, 4 * NH], bf16, tag="m")
                    m3 = m[:].rearrange("p (l i) -> p l i", l=4)
                    for o, n in CHUNKS:
                        g = gapool.tile([P, 4 * 896], bf16, tag="g")
                        c0 = k * 256 + h * 128 + o // 16
                        nc.gpsimd.dma_gather(
                            g[:, :4 * n].rearrange("p (j i) -> p j i", j=4),
                            xt_d.ap(),
                            idx16[:, c0:c0 + n // 16],
                            num_idxs=n, num_idxs_reg=n, elem_size=4 * P,
                            transpose=True)
                        nc.vector.tensor_tensor(
                            out=m3[:, :, o:o + n],
                            in0=g[:, :4 * n].rearrange(
                                "p (l i) -> p l i", l=4),
                            in1=wbc4[:, :, h * NH + o:h * NH + o + n],
                            op=ALU.mult)
                    for q in range(2):
                        for l in range(4):
                            for b2 in range(2):
                                c0 = b2 * 512
                                nc.tensor.matmul(
                                    ps[h][q][:, c0:c0 + 512], lhsT,
                                    m[:, l * NH + q * NQ + c0:
                                      l * NH + q * NQ + c0 + 512],
                                    start=(k == 0 and l == 0),
                                    stop=(k == K - 1 and l == 3),
                                    skip_group_check=True)

            # ---------------- stage 3: bias + store -------------------------
            for h in range(2):
                for q in range(2):
                    o = opool.tile([P, NQ], f32, tag="o")
                    nc.vector.tensor_scalar(
                        out=o[:], in0=ps[h][q][:],
                        scalar1=bias[:, 0:1], scalar2=None, op0=ALU.add)
                    p0 = h * NH + q * NQ
                    nc.scalar.dma_start(out=out_d.ap()[:, p0:p0 + NQ],
                                        in_=o[:])

    nc.compile()
    return nc


_NC_CACHE = None


def _host_inputs(x, offset, weight, bias):
    """Per-core input maps (core b <- batch b) + replicated constants."""
    wq = np.ascontiguousarray(weight, np.float32)  # [COUT, CIN, KH, KW]
    # wmat[c, k*COUT + o] = weight[o, c, ky, kx]
    wmat = wq.reshape(COUT, CIN, K).transpose(1, 2, 0).reshape(CIN, K * COUT)
    wmat = np.ascontiguousarray(wmat).astype(ml_dtypes.bfloat16)
    bias_h = np.ascontiguousarray(bias, np.float32).reshape(P, 1)
    # tab2[Pp, axis*288 + k*Q + q]: y: ho - 1 + ky + FB; x: wo - 1 + kx + FB
    p_of = (np.arange(Q)[:, None] * P + np.arange(P)[None, :])  # [Q, P]
    ho = (p_of // WO).astype(np.float32)
    wo = (p_of % WO).astype(np.float32)
    tab2 = np.empty((P, 2 * K * Q), np.float32)
    for k in range(K):
        tab2[:, k * Q:(k + 1) * Q] = (ho + (k // 3 - 1) + FB).T
        tab2[:, K * Q + k * Q:K * Q + (k + 1) * Q] = (wo + (k % 3 - 1) + FB).T
    in_maps = []
    for b in range(B):
        # off2[Pp, axis*288 + k*Q + q] = offset[b, 2k+axis, pos q*128+Pp]
        ob = np.ascontiguousarray(offset[b], np.float32).reshape(2 * K, NPOS)
        off2 = np.empty((P, 2 * K * Q), np.float32)
        oy = ob[0::2].reshape(K, Q, P)  # [k, q, Pp]
        ox = ob[1::2].reshape(K, Q, P)
        off2[:, :K * Q] = oy.transpose(2, 0, 1).reshape(P, K * Q)
        off2[:, K * Q:] = ox.transpose(2, 0, 1).reshape(P, K * Q)
        # xt4: padded position-major bf16, 4 corner rows packed per entry
        xp = np.zeros((HP, WP, P), np.float32)
        xp[PADR:PADR + H, PADR:PADR + W, :] = (
            np.ascontiguousarray(x[b], np.float32).transpose(1, 2, 0))
        flat = np.zeros((NE + WP + 2, P), np.float32)
        flat[:NE] = xp.reshape(NE, P)
        xt = np.empty((NE, 4 * P), np.float32)
        xt[:, 0 * P:1 * P] = flat[0:NE]
        xt[:, 1 * P:2 * P] = flat[1:NE + 1]
        xt[:, 2 * P:3 * P] = flat[WP:NE + WP]
        xt[:, 3 * P:4 * P] = flat[WP + 1:NE + WP + 1]
        in_maps.append({
            "xt": xt.astype(ml_dtypes.bfloat16),
            "off2": off2,
            "tab2": tab2,
            "wmat": wmat,
            "bias": bias_h,
        })
    return in_maps


def kernel(x, offset, weight, bias):
    global _NC_CACHE
    from concourse.bass_utils import run_bass_kernel_spmd

    if _NC_CACHE is None:
        _NC_CACHE = _build_kernel()
    nc = _NC_CACHE
    in_maps = _host_inputs(x, offset, weight, bias)
    res = run_bass_kernel_spmd(nc, in_maps, list(range(B)))
    out = np.stack([res.results[b]["out"].reshape(COUT, HO, WO) for b in range(B)])
    return out.astype(np.float32)


if __name__ == "__main__":
    import sys
    d = np.load("/tmp/inputs.npz")
    if len(sys.argv) > 1 and sys.argv[1] == "sim":
        from concourse.bass_interp import CoreSim
        nc = _build_kernel()
        in_maps = _host_inputs(d["x"], d["offset"], d["weight"], d["bias"])
        sim = CoreSim(nc)
        for kk, vv in in_maps[0].items():
            sim.tensor(kk)[:] = vv
        sim.simulate()
        out = np.asarray(sim.tensor("out")).reshape(1, COUT, HO, WO)
        exp = np.load("/tmp/expected.npy")[0:1]
    else:
        out = kernel(d["x"], d["offset"], d["weight"], d["bias"])
        exp = np.load("/tmp/expected.npy")
    err = np.abs(out - exp)
    print("rel l2:", np.linalg.norm(out - exp) / np.linalg.norm(exp))
    print("absmax rel:", err.max() / np.abs(exp).max())


# revision 13
# speedup vs baseline: 2.1955x; 2.1955x over previous
"""DeformConv2d (B=8, C=128, H=W=64, K=3x3, pad 1, stride 1) on 8 trn2 NeuronCores.

Data-parallel over batch: core b handles image b. Per core:
  - The padded image lives in DRAM position-major with all 4 bilinear corner
    rows packed per entry: xt4[r] = [pos r | r+1 | r+68 | r+69], each 128ch
    bf16, r = y*68+x over a 68x68 zero-ringed grid (ring width 2). One
    dma_gather index fetches all 4 corners of one sample point for all 128
    channels, transposed into channel-on-partition SBUF layout [c, l, i]
    (l = corner lane y0x0,y0x1,y1x0,y1x1).
  - Gather indices are computed by DVE directly in the SWDGE wrapped-16
    layout ([16, 2304], host-wrapped offset/base inputs), in 3 tap-group
    chunks so early gathers overlap the rest of the chain.
  - Bilinear corner weights are computed on DVE in natural [128, 288] layout,
    XBAR-transposed per tap ([128 pos, 128 (l,q)] -> [128 (l,q), 128 pos]),
    bounced to DRAM (contiguous), and broadcast to all 128 partitions with a
    stride-0-source DMA (one 32KB broadcast per tap, alternating HWDGE
    engines).
  - DVE multiplies gathered corners by broadcast weights (bf16); PE matmuls
    accumulate 9 taps x 4 corner lanes into per-quarter PSUM [128, 1024].
  - Tail: psum + bias -> fp32 output quarter, DMA to DRAM.
dma_gather note: transpose-mode gathers hang above ~1024 descriptors in
flight (SWDGE ring capacity); chunks are capped at 896 indices per call.
"""
import numpy as np
import ml_dtypes

B, CIN, H, W = 8, 128, 64, 64
COUT, KH, KW = 128, 3, 3
K = KH * KW
HO, WO = 64, 64
P = 128
NPOS = HO * WO              # 4096 positions
Q = NPOS // P               # 32 idx-cols in natural [128, K*Q] layout
PADR = 2
WP = W + 2 * PADR           # 68
HP = H + 2 * PADR           # 68
NE = HP * WP                # 4624 padded positions
FB = 1024.0                 # bias to keep pre-floor coords positive
NH = NPOS // 2              # 2048 positions per half
NQ = NPOS // 4              # 1024 positions per psum quarter
IDXC = K * Q * 8            # 2304 wrapped idx cols (k, q, g)
CHUNKS = [(0, 896), (896, 896), (1792, 256)]  # per-half gather chunks
KC = 3                      # idx chain computed in 3 tap-group chunks


def _build_kernel():
    import concourse.bacc as bacc
    import concourse.mybir as mybir
    import concourse.tile as tile
    import concourse.library_config as library_config

    nc = bacc.Bacc("TRN2", target_bir_lowering=False, debug=False,
                   num_devices=8)
    f32, bf16, i16 = mybir.dt.float32, mybir.dt.bfloat16, mybir.dt.int16
    i32 = mybir.dt.int32
    ALU = mybir.AluOpType

    xt_d = nc.dram_tensor("xt", [NE, 4 * P], bf16, kind="ExternalInput")
    offw_d = nc.dram_tensor("offw", [16, 2 * IDXC], f32, kind="ExternalInput")
    tabw_d = nc.dram_tensor("tabw", [16, 2 * IDXC], f32, kind="ExternalInput")
    off_d = nc.dram_tensor("off2", [P, 2 * K * Q], f32, kind="ExternalInput")
    tab_d = nc.dram_tensor("tab2", [P, 2 * K * Q], f32, kind="ExternalInput")
    wmat_d = nc.dram_tensor("wmat", [P, K * COUT], bf16, kind="ExternalInput")
    bias_d = nc.dram_tensor("bias", [P, 1], f32, kind="ExternalInput")
    out_d = nc.dram_tensor("out", [P, NPOS], f32, kind="ExternalOutput")

    with tile.TileContext(nc) as tc:
        with tc.tile_pool(name="const", bufs=1) as cpool, \
             tc.tile_pool(name="gen", bufs=1) as gpool, \
             tc.tile_pool(name="tk", bufs=3) as tkpool, \
             tc.tile_pool(name="wbc", bufs=2) as wpool, \
             tc.tile_pool(name="gath", bufs=5) as gapool, \
             tc.tile_pool(name="mm", bufs=2) as mpool, \
             tc.tile_pool(name="outp", bufs=2) as opool, \
             tc.tile_pool(name="dramw", bufs=1, space="DRAM") as dpool, \
             tc.tile_pool(name="ps", bufs=1, space="PSUM") as pspool:

            wrow = dpool.tile([K, 4 * NPOS], mybir.dt.bfloat16)

            nc.gpsimd.load_library(library_config.mlp)

            # ---------------- stage 0: input loads --------------------------
            off2 = cpool.tile([P, 2 * K * Q], f32)
            nc.scalar.dma_start(out=off2[:], in_=off_d.ap())
            tab2 = cpool.tile([P, 2 * K * Q], f32)
            nc.scalar.dma_start(out=tab2[:], in_=tab_d.ap())
            wmat = cpool.tile([P, K * COUT], bf16)
            nc.scalar.dma_start(out=wmat[:], in_=wmat_d.ap())
            bias = cpool.tile([P, 1], f32)
            nc.scalar.dma_start(out=bias[:], in_=bias_d.ap())

            # exact floor robust to cast rounding mode (trunc sim / RN hw);
            # returns (floor, frac); frac=None skips the frac outputs
            def floor_frac(pool, pb, rows, cols, sfx, want_frac=True):
                i0 = pool.tile([rows, cols], i32, tag="ffi" + sfx,
                               name="ffi" + sfx)
                nc.vector.tensor_copy(out=i0[:], in_=pb)
                f0 = pool.tile([rows, cols], f32, tag="fff" + sfx,
                               name="fff" + sfx)
                nc.vector.tensor_copy(out=f0[:], in_=i0[:])
                lr = pool.tile([rows, cols], f32, tag="ffl" + sfx,
                               name="ffl" + sfx)
                nc.vector.tensor_tensor(out=lr[:], in0=pb, in1=f0[:],
                                        op=ALU.subtract)
                adj = pool.tile([rows, cols], f32, tag="ffa" + sfx,
                                name="ffa" + sfx)
                nc.vector.tensor_scalar(out=adj[:], in0=lr[:], scalar1=0.0,
                                        scalar2=None, op0=ALU.is_lt)
                fl = pool.tile([rows, cols], f32, tag="ffo" + sfx,
                               name="ffo" + sfx)
                nc.vector.tensor_tensor(out=fl[:], in0=f0[:], in1=adj[:],
                                        op=ALU.subtract)
                if not want_frac:
                    return fl, None
                fr = pool.tile([rows, cols], f32, tag="ffr" + sfx,
                               name="ffr" + sfx)
                nc.vector.tensor_tensor(out=fr[:], in0=lr[:], in1=adj[:],
                                        op=ALU.add)
                return fl, fr

            # ------------- stage 1a: wrapped gather indices (3 chunks) ------
            # computed by DVE directly in the SWDGE wrapped-16 layout, with a
            # small rotating scratch set; per-chunk DMA loads of the wrapped
            # offsets/tables keep SBUF footprint low
            idx16 = gpool.tile([P, IDXC], i16)
            CW = IDXC // KC  # 768 cols per chunk

            def wchain_axis(offc, tabc, sfx):
                pw = gpool.tile([16, CW], f32, tag="wp" + sfx,
                                name="wp" + sfx)
                nc.vector.tensor_tensor(out=pw[:], in0=offc, in1=tabc,
                                        op=ALU.add)
                i0 = gpool.tile([16, CW], i32, tag="wi", name="wi")
                nc.vector.tensor_copy(out=i0[:], in_=pw[:])
                f0 = gpool.tile([16, CW], f32, tag="wf" + sfx,
                                name="wf" + sfx)
                nc.vector.tensor_copy(out=f0[:], in_=i0[:])
                lr = gpool.tile([16, CW], f32, tag="wl", name="wl")
                nc.vector.tensor_tensor(out=lr[:], in0=pw[:], in1=f0[:],
                                        op=ALU.subtract)
                adj = gpool.tile([16, CW], f32, tag="wa", name="wa")
                nc.vector.tensor_scalar(out=adj[:], in0=lr[:], scalar1=0.0,
                                        scalar2=None, op0=ALU.is_lt)
                fl = gpool.tile([16, CW], f32, tag="wp" + sfx,
                                name="wfl" + sfx)
                nc.vector.tensor_tensor(out=fl[:], in0=f0[:], in1=adj[:],
                                        op=ALU.subtract)
                cl = gpool.tile([16, CW], f32, tag="wf" + sfx,
                                name="wcl" + sfx)
                nc.vector.tensor_scalar(out=cl[:], in0=fl[:],
                                        scalar1=FB - PADR, scalar2=FB + 64.0,
                                        op0=ALU.max, op1=ALU.min)
                return cl

            for c in range(KC):
                cs = slice(c * CW, (c + 1) * CW)
                cs2 = slice(IDXC + c * CW, IDXC + (c + 1) * CW)
                ofy = gpool.tile([16, CW], f32, tag="ofy", name="ofy")
                nc.sync.dma_start(out=ofy[:], in_=offw_d.ap()[:, cs])
                tby = gpool.tile([16, CW], f32, tag="tby", name="tby")
                nc.sync.dma_start(out=tby[:], in_=tabw_d.ap()[:, cs])
                ofx = gpool.tile([16, CW], f32, tag="ofx", name="ofx")
                nc.scalar.dma_start(out=ofx[:], in_=offw_d.ap()[:, cs2])
                tbx = gpool.tile([16, CW], f32, tag="tbx", name="tbx")
                nc.scalar.dma_start(out=tbx[:], in_=tabw_d.ap()[:, cs2])
                ycw = wchain_axis(ofy[:], tby[:], "y")
                xcw = wchain_axis(ofx[:], tbx[:], "x")
                rfw = gpool.tile([16, CW], f32, tag="wl", name="rfw")
                nc.vector.scalar_tensor_tensor(
                    out=rfw[:], in0=ycw[:], scalar=float(WP), in1=xcw[:],
                    op0=ALU.mult, op1=ALU.add)
                nc.vector.tensor_scalar(
                    out=idx16[0:16, cs], in0=rfw[:],
                    scalar1=-(WP + 1.0) * FB + 2 * WP + 2.0,
                    scalar2=None, op0=ALU.add)
                # replicate to all 128 partitions (16 -> 32 -> 64 -> 128)
                nc.sync.dma_start(out=idx16[16:32, cs], in_=idx16[0:16, cs])
                nc.scalar.dma_start(out=idx16[32:64, cs], in_=idx16[0:32, cs])
                nc.sync.dma_start(out=idx16[64:128, cs], in_=idx16[0:64, cs])

            # ------------- stage 1b: bilinear weights + per-tap staging -----
            NG = K * Q  # 288
            py = gpool.tile([P, NG], f32)
            px = gpool.tile([P, NG], f32)
            nc.vector.tensor_tensor(out=py[:], in0=off2[:, 0:NG],
                                    in1=tab2[:, 0:NG], op=ALU.add)
            nc.vector.tensor_tensor(out=px[:], in0=off2[:, NG:2 * NG],
                                    in1=tab2[:, NG:2 * NG], op=ALU.add)
            _, ly = floor_frac(gpool, py[:], P, NG, "y")
            _, lx = floor_frac(gpool, px[:], P, NG, "x")
            omy = gpool.tile([P, NG], f32)
            omx = gpool.tile([P, NG], f32)
            nc.vector.tensor_scalar(out=omy[:], in0=ly[:], scalar1=-1.0,
                                    scalar2=1.0, op0=ALU.mult, op1=ALU.add)
            nc.vector.tensor_scalar(out=omx[:], in0=lx[:], scalar1=-1.0,
                                    scalar2=1.0, op0=ALU.mult, op1=ALU.add)
            # products laid out [Pp, (k, l, q)]; per-tap col block = (l, q)
            wpre = gpool.tile([P, K * 4 * Q], bf16)
            wv = wpre[:].rearrange("p (k l q) -> p k l q", k=K, l=4, q=Q)
            omy3 = omy[:].rearrange("p (k q) -> p k q", k=K, q=Q)
            ly3 = ly[:].rearrange("p (k q) -> p k q", k=K, q=Q)
            omx3 = omx[:].rearrange("p (k q) -> p k q", k=K, q=Q)
            lx3 = lx[:].rearrange("p (k q) -> p k q", k=K, q=Q)
            nc.vector.tensor_tensor(out=wv[:, :, 0], in0=omy3, in1=omx3,
                                    op=ALU.mult)  # y0 x0
            nc.vector.tensor_tensor(out=wv[:, :, 1], in0=omy3, in1=lx3,
                                    op=ALU.mult)  # y0 x1
            nc.vector.tensor_tensor(out=wv[:, :, 2], in0=ly3, in1=omx3,
                                    op=ALU.mult)  # y1 x0
            nc.vector.tensor_tensor(out=wv[:, :, 3], in0=ly3, in1=lx3,
                                    op=ALU.mult)  # y1 x1
            # per tap: XBAR transpose [128 Pp, 128 (l,q)] -> [128 (l,q), Pp],
            # then contiguous bounce to DRAM row (l, q, Pp) for broadcast
            for k in range(K):
                tk = tkpool.tile([P, P], bf16, tag="tk")
                nc.sync.dma_start(out=tk[:], in_=wpre[:, k * P:(k + 1) * P],
                                  transpose=True)
                nc.scalar.dma_start(
                    out=wrow[k:k + 1, :].rearrange("o (a b) -> (o a) b", b=P),
                    in_=tk[:])

            # ---------------- stage 2: per-tap gather/mult/matmul -----------
            # psum quarters: (h, q) -> positions [h*2048 + q*1024, +1024)
            ps = [[None, None], [None, None]]
            for h in range(2):
                for q in range(2):
                    psq = pspool.tile([P, NQ], f32, tag=f"ps{h}{q}",
                                      name=f"ps{h}{q}")
                    ps[h][q] = psq

            for k in range(K):
                wbc = wpool.tile([P, 4 * NPOS], bf16, tag="wb")
                beng = nc.sync if k % 2 == 0 else nc.scalar
                beng.dma_start(
                    out=wbc[:],
                    in_=wrow[k:k + 1, :].to_broadcast((P, 4 * NPOS)))
                wbc4 = wbc[:].rearrange("p (l i) -> p l i", l=4)
                lhsT = wmat[:, k * COUT:(k + 1) * COUT]
                for h in range(2):
                    m = mpool.tile([P, 4 * NH], bf16, tag="m")
                    m3 = m[:].rearrange("p (l i) -> p l i", l=4)
                    for o, n in CHUNKS:
                        g = gapool.tile([P, 4 * 896], bf16, tag="g")
                        c0 = k * 256 + h * 128 + o // 16
                        nc.gpsimd.dma_gather(
                            g[:, :4 * n].rearrange("p (j i) -> p j i", j=4),
                            xt_d.ap(),
                            idx16[:, c0:c0 + n // 16],
                            num_idxs=n, num_idxs_reg=n, elem_size=4 * P,
                            transpose=True)
                        nc.vector.tensor_tensor(
                            out=m3[:, :, o:o + n],
                            in0=g[:, :4 * n].rearrange(
                                "p (l i) -> p l i", l=4),
                            in1=wbc4[:, :, h * NH + o:h * NH + o + n],
                            op=ALU.mult)
                    for q in range(2):
                        for l in range(4):
                            for b2 in range(2):
                                c0 = b2 * 512
                                nc.tensor.matmul(
                                    ps[h][q][:, c0:c0 + 512], lhsT,
                                    m[:, l * NH + q * NQ + c0:
                                      l * NH + q * NQ + c0 + 512],
                                    start=(k == 0 and l == 0),
                                    stop=(k == K - 1 and l == 3),
                                    skip_group_check=True)

            # ---------------- stage 3: bias + store -------------------------
            for h in range(2):
                for q in range(2):
                    o = opool.tile([P, NQ], f32, tag="o")
                    nc.vector.tensor_scalar(
                        out=o[:], in0=ps[h][q][:],
                        scalar1=bias[:, 0:1], scalar2=None, op0=ALU.add)
                    p0 = h * NH + q * NQ
                    nc.scalar.dma_start(out=out_d.ap()[:, p0:p0 + NQ],
                                        in_=o[:])

    nc.compile()
    return nc


_NC_CACHE = None


def _host_inputs(x, offset, weight, bias):
    """Per-core input maps (core b <- batch b) + replicated constants."""
    wq = np.ascontiguousarray(weight, np.float32)  # [COUT, CIN, KH, KW]
    # wmat[c, k*COUT + o] = weight[o, c, ky, kx]
    wmat = wq.reshape(COUT, CIN, K).transpose(1, 2, 0).reshape(CIN, K * COUT)
    wmat = np.ascontiguousarray(wmat).astype(ml_dtypes.bfloat16)
    bias_h = np.ascontiguousarray(bias, np.float32).reshape(P, 1)
    # natural tables [Pp, axis*288 + k*Q + q] (pos p = q*128 + Pp)
    p_of = (np.arange(Q)[:, None] * P + np.arange(P)[None, :])  # [Q, P]
    ho = (p_of // WO).astype(np.float32)
    wo = (p_of % WO).astype(np.float32)
    tab2 = np.empty((P, 2 * K * Q), np.float32)
    for k in range(K):
        tab2[:, k * Q:(k + 1) * Q] = (ho + (k // 3 - 1) + FB).T
        tab2[:, K * Q + k * Q:K * Q + (k + 1) * Q] = (wo + (k % 3 - 1) + FB).T
    # wrapped tables [r, axis*2304 + k*256 + q*8 + g] (pos p = q*128+g*16+r)
    rr = np.arange(16)[:, None, None, None]
    kk = np.arange(K)[None, :, None, None]
    qq = np.arange(Q)[None, None, :, None]
    gg = np.arange(8)[None, None, None, :]
    pw = qq * P + gg * 16 + rr                      # [16, K, Q, 8]
    how = (pw // WO).astype(np.float32)
    wow = (pw % WO).astype(np.float32)
    tyw = how + (kk // 3 - 1) + FB
    txw = wow + (kk % 3 - 1) + FB
    tabw = np.concatenate(
        [tyw.reshape(16, IDXC), txw.reshape(16, IDXC)], axis=1
    ).astype(np.float32)
    in_maps = []
    for b in range(B):
        ob = np.ascontiguousarray(offset[b], np.float32).reshape(2 * K, NPOS)
        # natural off2[Pp, axis*288 + k*Q + q] = offset[2k+axis, q*128+Pp]
        off2 = np.empty((P, 2 * K * Q), np.float32)
        oy = ob[0::2].reshape(K, Q, P)  # [k, q, Pp]
        ox = ob[1::2].reshape(K, Q, P)
        off2[:, :K * Q] = oy.transpose(2, 0, 1).reshape(P, K * Q)
        off2[:, K * Q:] = ox.transpose(2, 0, 1).reshape(P, K * Q)
        # wrapped offw[r, axis*2304 + k*256 + q*8 + g]
        oyw = oy.reshape(K, Q, 8, 16)   # [k, q, g, r]
        oxw = ox.reshape(K, Q, 8, 16)
        offw = np.concatenate(
            [oyw.transpose(3, 0, 1, 2).reshape(16, IDXC),
             oxw.transpose(3, 0, 1, 2).reshape(16, IDXC)], axis=1
        ).astype(np.float32)
        # xt4: padded position-major bf16, 4 corner rows packed per entry
        xp = np.zeros((HP, WP, P), np.float32)
        xp[PADR:PADR + H, PADR:PADR + W, :] = (
            np.ascontiguousarray(x[b], np.float32).transpose(1, 2, 0))
        flat = np.zeros((NE + WP + 2, P), np.float32)
        flat[:NE] = xp.reshape(NE, P)
        xt = np.empty((NE, 4 * P), np.float32)
        xt[:, 0 * P:1 * P] = flat[0:NE]
        xt[:, 1 * P:2 * P] = flat[1:NE + 1]
        xt[:, 2 * P:3 * P] = flat[WP:NE + WP]
        xt[:, 3 * P:4 * P] = flat[WP + 1:NE + WP + 1]
        in_maps.append({
            "xt": xt.astype(ml_dtypes.bfloat16),
            "offw": offw,
            "tabw": tabw,
            "off2": off2,
            "tab2": tab2,
            "wmat": wmat,
            "bias": bias_h,
        })
    return in_maps


def kernel(x, offset, weight, bias):
    global _NC_CACHE
    from concourse.bass_utils import run_bass_kernel_spmd

    if _NC_CACHE is None:
        _NC_CACHE = _build_kernel()
    nc = _NC_CACHE
    in_maps = _host_inputs(x, offset, weight, bias)
    res = run_bass_kernel_spmd(nc, in_maps, list(range(B)))
    out = np.stack([res.results[b]["out"].reshape(COUT, HO, WO) for b in range(B)])
    return out.astype(np.float32)


if __name__ == "__main__":
    import sys
    d = np.load("/tmp/inputs.npz")
    if len(sys.argv) > 1 and sys.argv[1] == "sim":
        from concourse.bass_interp import CoreSim
        nc = _build_kernel()
        in_maps = _host_inputs(d["x"], d["offset"], d["weight"], d["bias"])
        sim = CoreSim(nc)
        for kk, vv in in_maps[0].items():
            sim.tensor(kk)[:] = vv
        sim.simulate()
        out = np.asarray(sim.tensor("out")).reshape(1, COUT, HO, WO)
        exp = np.load("/tmp/expected.npy")[0:1]
    else:
        out = kernel(d["x"], d["offset"], d["weight"], d["bias"])
        exp = np.load("/tmp/expected.npy")
    err = np.abs(out - exp)
    print("rel l2:", np.linalg.norm(out - exp) / np.linalg.norm(exp))
    print("absmax rel:", err.max() / np.abs(exp).max())


# revision 15
# speedup vs baseline: 2.3806x; 1.0843x over previous
"""DeformConv2d (B=8, C=128, H=W=64, K=3x3, pad 1, stride 1) on 8 trn2 NeuronCores.

Data-parallel over batch: core b handles image b. Per core:
  - The padded image lives in DRAM position-major with all 4 bilinear corner
    rows packed per entry: xt4[r] = [pos r | r+1 | r+68 | r+69], each 128ch
    bf16, r = y*68+x over a 68x68 zero-ringed grid (ring width 2). One
    dma_gather index fetches all 4 corners of one sample point for all 128
    channels, transposed into channel-on-partition SBUF layout [c, l, i]
    (l = corner lane y0x0,y0x1,y1x0,y1x1).
  - Gather indices are computed by DVE directly in the SWDGE wrapped-16
    layout ([16, 2304], host-wrapped offset/base inputs), in 3 tap-group
    chunks so early gathers overlap the rest of the chain.
  - Bilinear corner weights are computed on DVE in natural [128, 288] layout,
    XBAR-transposed per tap ([128 pos, 128 (l,q)] -> [128 (l,q), 128 pos]),
    bounced to DRAM (contiguous), and broadcast to all 128 partitions with a
    stride-0-source DMA (one 32KB broadcast per tap, alternating HWDGE
    engines).
  - DVE multiplies gathered corners by broadcast weights (bf16); PE matmuls
    accumulate 9 taps x 4 corner lanes into per-quarter PSUM [128, 1024].
  - Tail: psum + bias -> fp32 output quarter, DMA to DRAM.
dma_gather note: transpose-mode gathers hang above ~1024 descriptors in
flight (SWDGE ring capacity); chunks are capped at 896 indices per call.
"""
import numpy as np
import ml_dtypes

B, CIN, H, W = 8, 128, 64, 64
COUT, KH, KW = 128, 3, 3
K = KH * KW
HO, WO = 64, 64
P = 128
NPOS = HO * WO              # 4096 positions
Q = NPOS // P               # 32 idx-cols in natural [128, K*Q] layout
PADR = 2
WP = W + 2 * PADR           # 68
HP = H + 2 * PADR           # 68
NE = HP * WP                # 4624 padded positions
FB = 1024.0                 # bias to keep pre-floor coords positive
NH = NPOS // 2              # 2048 positions per half
NQ = NPOS // 4              # 1024 positions per psum quarter
IDXC = K * Q * 8            # 2304 wrapped idx cols (k, q, g)
CHUNKS = [(0, 896), (896, 896), (1792, 256)]  # per-half gather chunks
KC = 3                      # idx chain computed in 3 tap-group chunks


def _build_kernel():
    import concourse.bacc as bacc
    import concourse.mybir as mybir
    import concourse.tile as tile
    import concourse.library_config as library_config

    nc = bacc.Bacc("TRN2", target_bir_lowering=False, debug=False,
                   num_devices=8)
    f32, bf16, i16 = mybir.dt.float32, mybir.dt.bfloat16, mybir.dt.int16
    i32 = mybir.dt.int32
    ALU = mybir.AluOpType

    xt_d = nc.dram_tensor("xt", [NE, 4 * P], bf16, kind="ExternalInput")
    offw_d = nc.dram_tensor("offw", [16, 2 * IDXC], f32, kind="ExternalInput")
    tabw_d = nc.dram_tensor("tabw", [16, 2 * IDXC], f32, kind="ExternalInput")
    off_d = nc.dram_tensor("off2", [P, 2 * K * Q], f32, kind="ExternalInput")
    tab_d = nc.dram_tensor("tab2", [P, 2 * K * Q], f32, kind="ExternalInput")
    wmat_d = nc.dram_tensor("wmat", [P, K * COUT], bf16, kind="ExternalInput")
    bias_d = nc.dram_tensor("bias", [P, 1], f32, kind="ExternalInput")
    out_d = nc.dram_tensor("out", [P, NPOS], f32, kind="ExternalOutput")

    with tile.TileContext(nc) as tc:
        with tc.tile_pool(name="const", bufs=1) as cpool, \
             tc.tile_pool(name="gen", bufs=1) as gpool, \
             tc.tile_pool(name="tk", bufs=3) as tkpool, \
             tc.tile_pool(name="wbc", bufs=2) as wpool, \
             tc.tile_pool(name="gath", bufs=5) as gapool, \
             tc.tile_pool(name="mm", bufs=2) as mpool, \
             tc.tile_pool(name="outp", bufs=2) as opool, \
             tc.tile_pool(name="dramw", bufs=1, space="DRAM") as dpool, \
             tc.tile_pool(name="ps", bufs=1, space="PSUM") as pspool:

            wrow = dpool.tile([K, 4 * NPOS], mybir.dt.bfloat16)

            nc.gpsimd.load_library(library_config.mlp)

            # ---------------- stage 0: input loads --------------------------
            off2 = cpool.tile([P, 2 * K * Q], f32)
            nc.scalar.dma_start(out=off2[:], in_=off_d.ap())
            tab2 = cpool.tile([P, 2 * K * Q], f32)
            nc.scalar.dma_start(out=tab2[:], in_=tab_d.ap())
            wmat = cpool.tile([P, K * COUT], bf16)
            nc.scalar.dma_start(out=wmat[:], in_=wmat_d.ap())
            bias = cpool.tile([P, 1], f32)
            nc.scalar.dma_start(out=bias[:], in_=bias_d.ap())

            # exact floor robust to cast rounding mode (trunc sim / RN hw);
            # returns (floor, frac); frac=None skips the frac outputs
            def floor_frac(pool, pb, rows, cols, sfx, want_frac=True):
                i0 = pool.tile([rows, cols], i32, tag="ffi" + sfx,
                               name="ffi" + sfx)
                nc.scalar.copy(out=i0[:], in_=pb)
                f0 = pool.tile([rows, cols], f32, tag="fff" + sfx,
                               name="fff" + sfx)
                nc.scalar.copy(out=f0[:], in_=i0[:])
                lr = pool.tile([rows, cols], f32, tag="ffl" + sfx,
                               name="ffl" + sfx)
                nc.vector.tensor_tensor(out=lr[:], in0=pb, in1=f0[:],
                                        op=ALU.subtract)
                adj = pool.tile([rows, cols], f32, tag="ffa" + sfx,
                                name="ffa" + sfx)
                nc.vector.tensor_scalar(out=adj[:], in0=lr[:], scalar1=0.0,
                                        scalar2=None, op0=ALU.is_lt)
                fl = pool.tile([rows, cols], f32, tag="ffo" + sfx,
                               name="ffo" + sfx)
                nc.vector.tensor_tensor(out=fl[:], in0=f0[:], in1=adj[:],
                                        op=ALU.subtract)
                if not want_frac:
                    return fl, None
                fr = pool.tile([rows, cols], f32, tag="ffr" + sfx,
                               name="ffr" + sfx)
                nc.vector.tensor_tensor(out=fr[:], in0=lr[:], in1=adj[:],
                                        op=ALU.add)
                return fl, fr

            # ------------- stage 1b: bilinear weights + per-tap staging -----
            NG = K * Q  # 288
            py = gpool.tile([P, NG], f32)
            px = gpool.tile([P, NG], f32)
            nc.vector.tensor_tensor(out=py[:], in0=off2[:, 0:NG],
                                    in1=tab2[:, 0:NG], op=ALU.add)
            nc.vector.tensor_tensor(out=px[:], in0=off2[:, NG:2 * NG],
                                    in1=tab2[:, NG:2 * NG], op=ALU.add)
            _, ly = floor_frac(gpool, py[:], P, NG, "y")
            _, lx = floor_frac(gpool, px[:], P, NG, "x")
            omy = gpool.tile([P, NG], f32)
            omx = gpool.tile([P, NG], f32)
            nc.vector.tensor_scalar(out=omy[:], in0=ly[:], scalar1=-1.0,
                                    scalar2=1.0, op0=ALU.mult, op1=ALU.add)
            nc.vector.tensor_scalar(out=omx[:], in0=lx[:], scalar1=-1.0,
                                    scalar2=1.0, op0=ALU.mult, op1=ALU.add)
            # products laid out [Pp, (k, l, q)]; per-tap col block = (l, q)
            wpre = gpool.tile([P, K * 4 * Q], bf16)
            wv = wpre[:].rearrange("p (k l q) -> p k l q", k=K, l=4, q=Q)
            omy3 = omy[:].rearrange("p (k q) -> p k q", k=K, q=Q)
            ly3 = ly[:].rearrange("p (k q) -> p k q", k=K, q=Q)
            omx3 = omx[:].rearrange("p (k q) -> p k q", k=K, q=Q)
            lx3 = lx[:].rearrange("p (k q) -> p k q", k=K, q=Q)
            nc.vector.tensor_tensor(out=wv[:, :, 0], in0=omy3, in1=omx3,
                                    op=ALU.mult)  # y0 x0
            nc.vector.tensor_tensor(out=wv[:, :, 1], in0=omy3, in1=lx3,
                                    op=ALU.mult)  # y0 x1
            nc.vector.tensor_tensor(out=wv[:, :, 2], in0=ly3, in1=omx3,
                                    op=ALU.mult)  # y1 x0
            nc.vector.tensor_tensor(out=wv[:, :, 3], in0=ly3, in1=lx3,
                                    op=ALU.mult)  # y1 x1
            # per tap: XBAR transpose [128 Pp, 128 (l,q)] -> [128 (l,q), Pp],
            # then contiguous bounce to DRAM row (l, q, Pp) for broadcast
            for k in range(K):
                tk = tkpool.tile([P, P], bf16, tag="tk")
                nc.sync.dma_start(out=tk[:], in_=wpre[:, k * P:(k + 1) * P],
                                  transpose=True)
                nc.scalar.dma_start(
                    out=wrow[k:k + 1, :].rearrange("o (a b) -> (o a) b", b=P),
                    in_=tk[:])

            # ------------- stage 1a: wrapped gather indices (3 chunks) ------
            # computed by DVE directly in the SWDGE wrapped-16 layout, with a
            # small rotating scratch set; per-chunk DMA loads of the wrapped
            # offsets/tables keep SBUF footprint low
            idx16 = gpool.tile([P, IDXC], i16)
            CW = IDXC // KC  # 768 cols per chunk

            def wchain_axis(offc, tabc, sfx):
                pw = gpool.tile([16, CW], f32, tag="wp" + sfx,
                                name="wp" + sfx)
                nc.vector.tensor_tensor(out=pw[:], in0=offc, in1=tabc,
                                        op=ALU.add)
                i0 = gpool.tile([16, CW], i32, tag="wi", name="wi")
                nc.scalar.copy(out=i0[:], in_=pw[:])
                f0 = gpool.tile([16, CW], f32, tag="wf" + sfx,
                                name="wf" + sfx)
                nc.scalar.copy(out=f0[:], in_=i0[:])
                lr = gpool.tile([16, CW], f32, tag="wl", name="wl")
                nc.vector.tensor_tensor(out=lr[:], in0=pw[:], in1=f0[:],
                                        op=ALU.subtract)
                adj = gpool.tile([16, CW], f32, tag="wa", name="wa")
                nc.vector.tensor_scalar(out=adj[:], in0=lr[:], scalar1=0.0,
                                        scalar2=None, op0=ALU.is_lt)
                fl = gpool.tile([16, CW], f32, tag="wp" + sfx,
                                name="wfl" + sfx)
                nc.vector.tensor_tensor(out=fl[:], in0=f0[:], in1=adj[:],
                                        op=ALU.subtract)
                cl = gpool.tile([16, CW], f32, tag="wf" + sfx,
                                name="wcl" + sfx)
                nc.vector.tensor_scalar(out=cl[:], in0=fl[:],
                                        scalar1=FB - PADR, scalar2=FB + 64.0,
                                        op0=ALU.max, op1=ALU.min)
                return cl

            for c in range(KC):
                cs = slice(c * CW, (c + 1) * CW)
                cs2 = slice(IDXC + c * CW, IDXC + (c + 1) * CW)
                ofy = gpool.tile([16, CW], f32, tag="ofy", name="ofy")
                nc.sync.dma_start(out=ofy[:], in_=offw_d.ap()[:, cs])
                tby = gpool.tile([16, CW], f32, tag="tby", name="tby")
                nc.sync.dma_start(out=tby[:], in_=tabw_d.ap()[:, cs])
                ofx = gpool.tile([16, CW], f32, tag="ofx", name="ofx")
                nc.scalar.dma_start(out=ofx[:], in_=offw_d.ap()[:, cs2])
                tbx = gpool.tile([16, CW], f32, tag="tbx", name="tbx")
                nc.scalar.dma_start(out=tbx[:], in_=tabw_d.ap()[:, cs2])
                ycw = wchain_axis(ofy[:], tby[:], "y")
                xcw = wchain_axis(ofx[:], tbx[:], "x")
                rfw = gpool.tile([16, CW], f32, tag="wl", name="rfw")
                nc.vector.scalar_tensor_tensor(
                    out=rfw[:], in0=ycw[:], scalar=float(WP), in1=xcw[:],
                    op0=ALU.mult, op1=ALU.add)
                nc.vector.tensor_scalar(
                    out=idx16[0:16, cs], in0=rfw[:],
                    scalar1=-(WP + 1.0) * FB + 2 * WP + 2.0,
                    scalar2=None, op0=ALU.add)
                # replicate to all 128 partitions (16 -> 32 -> 64 -> 128)
                nc.sync.dma_start(out=idx16[16:32, cs], in_=idx16[0:16, cs])
                nc.scalar.dma_start(out=idx16[32:64, cs], in_=idx16[0:32, cs])
                nc.sync.dma_start(out=idx16[64:128, cs], in_=idx16[0:64, cs])

            # ---------------- stage 2: per-tap gather/mult/matmul -----------
            # psum quarters: (h, q) -> positions [h*2048 + q*1024, +1024)
            ps = [[None, None], [None, None]]
            for h in range(2):
                for q in range(2):
                    psq = pspool.tile([P, NQ], f32, tag=f"ps{h}{q}",
                                      name=f"ps{h}{q}")
                    ps[h][q] = psq

            for k in range(K):
                wbc = wpool.tile([P, 4 * NPOS], bf16, tag="wb")
                beng = nc.sync if k % 2 == 0 else nc.scalar
                beng.dma_start(
                    out=wbc[:],
                    in_=wrow[k:k + 1, :].to_broadcast((P, 4 * NPOS)))
                wbc4 = wbc[:].rearrange("p (l i) -> p l i", l=4)
                lhsT = wmat[:, k * COUT:(k + 1) * COUT]
                for h in range(2):
                    m = mpool.tile([P, 4 * NH], bf16, tag="m")
                    m3 = m[:].rearrange("p (l i) -> p l i", l=4)
                    for o, n in CHUNKS:
                        g = gapool.tile([P, 4 * 896], bf16, tag="g")
                        c0 = k * 256 + h * 128 + o // 16
                        nc.gpsimd.dma_gather(
                            g[:, :4 * n].rearrange("p (j i) -> p j i", j=4),
                            xt_d.ap(),
                            idx16[:, c0:c0 + n // 16],
                            num_idxs=n, num_idxs_reg=n, elem_size=4 * P,
                            transpose=True, single_packet=False)
                        nc.vector.tensor_tensor(
                            out=m3[:, :, o:o + n],
                            in0=g[:, :4 * n].rearrange(
                                "p (l i) -> p l i", l=4),
                            in1=wbc4[:, :, h * NH + o:h * NH + o + n],
                            op=ALU.mult)
                    for q in range(2):
                        for l in range(4):
                            for b2 in range(2):
                                c0 = b2 * 512
                                nc.tensor.matmul(
                                    ps[h][q][:, c0:c0 + 512], lhsT,
                                    m[:, l * NH + q * NQ + c0:
                                      l * NH + q * NQ + c0 + 512],
                                    start=(k == 0 and l == 0),
                                    stop=(k == K - 1 and l == 3),
                                    skip_group_check=True)

            # ---------------- stage 3: bias + store -------------------------
            for h in range(2):
                for q in range(2):
                    o = opool.tile([P, NQ], f32, tag="o")
                    nc.vector.tensor_scalar(
                        out=o[:], in0=ps[h][q][:],
                        scalar1=bias[:, 0:1], scalar2=None, op0=ALU.add)
                    p0 = h * NH + q * NQ
                    nc.scalar.dma_start(out=out_d.ap()[:, p0:p0 + NQ],
                                        in_=o[:])

    nc.compile()
    return nc


_NC_CACHE = None


def _host_inputs(x, offset, weight, bias):
    """Per-core input maps (core b <- batch b) + replicated constants."""
    wq = np.ascontiguousarray(weight, np.float32)  # [COUT, CIN, KH, KW]
    # wmat[c, k*COUT + o] = weight[o, c, ky, kx]
    wmat = wq.reshape(COUT, CIN, K).transpose(1, 2, 0).reshape(CIN, K * COUT)
    wmat = np.ascontiguousarray(wmat).astype(ml_dtypes.bfloat16)
    bias_h = np.ascontiguousarray(bias, np.float32).reshape(P, 1)
    # natural tables [Pp, axis*288 + k*Q + q] (pos p = q*128 + Pp)
    p_of = (np.arange(Q)[:, None] * P + np.arange(P)[None, :])  # [Q, P]
    ho = (p_of // WO).astype(np.float32)
    wo = (p_of % WO).astype(np.float32)
    tab2 = np.empty((P, 2 * K * Q), np.float32)
    for k in range(K):
        tab2[:, k * Q:(k + 1) * Q] = (ho + (k // 3 - 1) + FB).T
        tab2[:, K * Q + k * Q:K * Q + (k + 1) * Q] = (wo + (k % 3 - 1) + FB).T
    # wrapped tables [r, axis*2304 + k*256 + q*8 + g] (pos p = q*128+g*16+r)
    rr = np.arange(16)[:, None, None, None]
    kk = np.arange(K)[None, :, None, None]
    qq = np.arange(Q)[None, None, :, None]
    gg = np.arange(8)[None, None, None, :]
    pw = qq * P + gg * 16 + rr                      # [16, K, Q, 8]
    how = (pw // WO).astype(np.float32)
    wow = (pw % WO).astype(np.float32)
    tyw = how + (kk // 3 - 1) + FB
    txw = wow + (kk % 3 - 1) + FB
    tabw = np.concatenate(
        [tyw.reshape(16, IDXC), txw.reshape(16, IDXC)], axis=1
    ).astype(np.float32)
    in_maps = []
    for b in range(B):
        ob = np.ascontiguousarray(offset[b], np.float32).reshape(2 * K, NPOS)
        # natural off2[Pp, axis*288 + k*Q + q] = offset[2k+axis, q*128+Pp]
        off2 = np.empty((P, 2 * K * Q), np.float32)
        oy = ob[0::2].reshape(K, Q, P)  # [k, q, Pp]
        ox = ob[1::2].reshape(K, Q, P)
        off2[:, :K * Q] = oy.transpose(2, 0, 1).reshape(P, K * Q)
        off2[:, K * Q:] = ox.transpose(2, 0, 1).reshape(P, K * Q)
        # wrapped offw[r, axis*2304 + k*256 + q*8 + g]
        oyw = oy.reshape(K, Q, 8, 16)   # [k, q, g, r]
        oxw = ox.reshape(K, Q, 8, 16)
        offw = np.concatenate(
            [oyw.transpose(3, 0, 1, 2).reshape(16, IDXC),
             oxw.transpose(3, 0, 1, 2).reshape(16, IDXC)], axis=1
        ).astype(np.float32)
        # xt4: padded position-major bf16, 4 corner rows packed per entry
        xp = np.zeros((HP, WP, P), np.float32)
        xp[PADR:PADR + H, PADR:PADR + W, :] = (
            np.ascontiguousarray(x[b], np.float32).transpose(1, 2, 0))
        flat = np.zeros((NE + WP + 2, P), np.float32)
        flat[:NE] = xp.reshape(NE, P)
        xt = np.empty((NE, 4 * P), np.float32)
        xt[:, 0 * P:1 * P] = flat[0:NE]
        xt[:, 1 * P:2 * P] = flat[1:NE + 1]
        xt[:, 2 * P:3 * P] = flat[WP:NE + WP]
        xt[:, 3 * P:4 * P] = flat[WP + 1:NE + WP + 1]
        in_maps.append({
            "xt": xt.astype(ml_dtypes.bfloat16),
            "offw": offw,
            "tabw": tabw,
            "off2": off2,
            "tab2": tab2,
            "wmat": wmat,
            "bias": bias_h,
        })
    return in_maps


def kernel(x, offset, weight, bias):
    global _NC_CACHE
    from concourse.bass_utils import run_bass_kernel_spmd

    if _NC_CACHE is None:
        _NC_CACHE = _build_kernel()
    nc = _NC_CACHE
    in_maps = _host_inputs(x, offset, weight, bias)
    res = run_bass_kernel_spmd(nc, in_maps, list(range(B)))
    out = np.stack([res.results[b]["out"].reshape(COUT, HO, WO) for b in range(B)])
    return out.astype(np.float32)


if __name__ == "__main__":
    import sys
    d = np.load("/tmp/inputs.npz")
    if len(sys.argv) > 1 and sys.argv[1] == "sim":
        from concourse.bass_interp import CoreSim
        nc = _build_kernel()
        in_maps = _host_inputs(d["x"], d["offset"], d["weight"], d["bias"])
        sim = CoreSim(nc)
        for kk, vv in in_maps[0].items():
            sim.tensor(kk)[:] = vv
        sim.simulate()
        out = np.asarray(sim.tensor("out")).reshape(1, COUT, HO, WO)
        exp = np.load("/tmp/expected.npy")[0:1]
    else:
        out = kernel(d["x"], d["offset"], d["weight"], d["bias"])
        exp = np.load("/tmp/expected.npy")
    err = np.abs(out - exp)
    print("rel l2:", np.linalg.norm(out - exp) / np.linalg.norm(exp))
    print("absmax rel:", err.max() / np.abs(exp).max())


# revision 21
# speedup vs baseline: 2.4763x; 1.0402x over previous
"""DeformConv2d (B=8, C=128, H=W=64, K=3x3, pad 1, stride 1) on 8 trn2 NeuronCores.

Data-parallel over batch: core b handles image b. Per core:
  - The padded image lives in DRAM position-major with all 4 bilinear corner
    rows packed per entry: xt4[r] = [pos r | r+1 | r+68 | r+69], each 128ch
    bf16, r = y*68+x over a 68x68 zero-ringed grid (ring width 2). One
    dma_gather index fetches all 4 corners of one sample point for all 128
    channels, transposed into channel-on-partition SBUF layout [c, l, i]
    (l = corner lane y0x0,y0x1,y1x0,y1x1).
  - Gather indices are computed by DVE directly in the SWDGE wrapped-16
    layout ([16, 2304], host-wrapped offset/base inputs), in 3 tap-group
    chunks so early gathers overlap the rest of the chain.
  - Bilinear corner weights are computed on DVE in natural [128, 288] layout,
    XBAR-transposed per tap ([128 pos, 128 (l,q)] -> [128 (l,q), 128 pos]),
    bounced to DRAM (contiguous), and broadcast to all 128 partitions with a
    stride-0-source DMA (one 32KB broadcast per tap, alternating HWDGE
    engines).
  - DVE multiplies gathered corners by broadcast weights (bf16); PE matmuls
    accumulate 9 taps x 4 corner lanes into per-quarter PSUM [128, 1024].
  - Tail: psum + bias -> fp32 output quarter, DMA to DRAM.
dma_gather note: transpose-mode gathers hang above ~1024 descriptors in
flight (SWDGE ring capacity); chunks are capped at 896 indices per call.
"""
import numpy as np
import ml_dtypes

B, CIN, H, W = 8, 128, 64, 64
COUT, KH, KW = 128, 3, 3
K = KH * KW
HO, WO = 64, 64
P = 128
NPOS = HO * WO              # 4096 positions
Q = NPOS // P               # 32 idx-cols in natural [128, K*Q] layout
PADR = 2
WP = W + 2 * PADR           # 68
HP = H + 2 * PADR           # 68
NE = HP * WP                # 4624 padded positions
FB = 1024.0                 # bias to keep pre-floor coords positive
NH = NPOS // 2              # 2048 positions per half
NQ = NPOS // 4              # 1024 positions per psum quarter
IDXC = K * Q * 8            # 2304 wrapped idx cols (k, q, g)
CHUNKS = [(0, 896), (896, 896), (1792, 256)]  # per-half gather chunks
KC = 3                      # idx chain computed in 3 tap-group chunks


def _build_kernel():
    import concourse.bacc as bacc
    import concourse.mybir as mybir
    import concourse.tile as tile
    import concourse.library_config as library_config

    nc = bacc.Bacc("TRN2", target_bir_lowering=False, debug=False,
                   num_devices=8)
    f32, bf16, i16 = mybir.dt.float32, mybir.dt.bfloat16, mybir.dt.int16
    i32 = mybir.dt.int32
    ALU = mybir.AluOpType

    xt_d = nc.dram_tensor("xt", [NE, 4 * P], bf16, kind="ExternalInput")
    offw_d = nc.dram_tensor("offw", [16, 2 * IDXC], f32, kind="ExternalInput")
    tabw_d = nc.dram_tensor("tabw", [16, 2 * IDXC], f32, kind="ExternalInput")
    off_d = nc.dram_tensor("off2", [P, 2 * K * Q], f32, kind="ExternalInput")
    tab_d = nc.dram_tensor("tab2", [P, 2 * K * Q], f32, kind="ExternalInput")
    wmat_d = nc.dram_tensor("wmat", [P, K * COUT], bf16, kind="ExternalInput")
    bias_d = nc.dram_tensor("bias", [P, 1], f32, kind="ExternalInput")
    out_d = nc.dram_tensor("out", [P, NPOS], f32, kind="ExternalOutput")

    with tile.TileContext(nc) as tc:
        with tc.tile_pool(name="const", bufs=1) as cpool, \
             tc.tile_pool(name="gen", bufs=1) as gpool, \
             tc.tile_pool(name="tk", bufs=3) as tkpool, \
             tc.tile_pool(name="wbc", bufs=2) as wpool, \
             tc.tile_pool(name="gath", bufs=3) as gapool, \
             tc.tile_pool(name="mm", bufs=2) as mpool, \
             tc.tile_pool(name="outp", bufs=1) as opool, \
             tc.tile_pool(name="dramw", bufs=1, space="DRAM") as dpool, \
             tc.tile_pool(name="ps", bufs=1, space="PSUM") as pspool:

            wrow = dpool.tile([K, 4 * NPOS], mybir.dt.bfloat16)

            nc.gpsimd.load_library(library_config.mlp)

            # ---------------- stage 0: input loads --------------------------
            off2 = cpool.tile([P, 2 * K * Q], f32)
            nc.scalar.dma_start(out=off2[:], in_=off_d.ap())
            tab2 = cpool.tile([P, 2 * K * Q], f32)
            nc.scalar.dma_start(out=tab2[:], in_=tab_d.ap())
            wmat = cpool.tile([P, K * COUT], bf16)
            nc.scalar.dma_start(out=wmat[:], in_=wmat_d.ap())
            bias = cpool.tile([P, 1], f32)
            nc.scalar.dma_start(out=bias[:], in_=bias_d.ap())

            # exact floor robust to cast rounding mode (trunc sim / RN hw);
            # returns (floor, frac); frac=None skips the frac outputs
            def floor_frac(pool, pb, rows, cols, sfx, want_frac=True):
                i0 = pool.tile([rows, cols], i32, tag="ffi" + sfx,
                               name="ffi" + sfx)
                nc.vector.tensor_copy(out=i0[:], in_=pb)
                f0 = pool.tile([rows, cols], f32, tag="fff" + sfx,
                               name="fff" + sfx)
                nc.vector.tensor_copy(out=f0[:], in_=i0[:])
                lr = pool.tile([rows, cols], f32, tag="ffl" + sfx,
                               name="ffl" + sfx)
                nc.vector.tensor_tensor(out=lr[:], in0=pb, in1=f0[:],
                                        op=ALU.subtract)
                adj = pool.tile([rows, cols], f32, tag="ffa" + sfx,
                                name="ffa" + sfx)
                nc.vector.tensor_scalar(out=adj[:], in0=lr[:], scalar1=0.0,
                                        scalar2=None, op0=ALU.is_lt)
                if want_frac:
                    fr = pool.tile([rows, cols], f32, tag="ffr" + sfx,
                                   name="ffr" + sfx)
                    nc.vector.tensor_tensor(out=fr[:], in0=lr[:], in1=adj[:],
                                            op=ALU.add)
                    return None, fr
                fl = pool.tile([rows, cols], f32, tag="ffo" + sfx,
                               name="ffo" + sfx)
                nc.vector.tensor_tensor(out=fl[:], in0=f0[:], in1=adj[:],
                                        op=ALU.subtract)
                return fl, None

            # ------------- stage 1b: bilinear weights + per-tap staging -----
            NG = K * Q  # 288
            py = gpool.tile([P, NG], f32)
            px = gpool.tile([P, NG], f32)
            nc.vector.tensor_tensor(out=py[:], in0=off2[:, 0:NG],
                                    in1=tab2[:, 0:NG], op=ALU.add)
            nc.vector.tensor_tensor(out=px[:], in0=off2[:, NG:2 * NG],
                                    in1=tab2[:, NG:2 * NG], op=ALU.add)
            _, ly = floor_frac(gpool, py[:], P, NG, "y")
            _, lx = floor_frac(gpool, px[:], P, NG, "x")
            omy = gpool.tile([P, NG], f32)
            omx = gpool.tile([P, NG], f32)
            nc.vector.tensor_scalar(out=omy[:], in0=ly[:], scalar1=-1.0,
                                    scalar2=1.0, op0=ALU.mult, op1=ALU.add)
            nc.vector.tensor_scalar(out=omx[:], in0=lx[:], scalar1=-1.0,
                                    scalar2=1.0, op0=ALU.mult, op1=ALU.add)
            # products laid out [Pp, (k, l, q)]; per-tap col block = (l, q)
            wpre = gpool.tile([P, K * 4 * Q], bf16)
            wv = wpre[:].rearrange("p (k l q) -> p k l q", k=K, l=4, q=Q)
            omy3 = omy[:].rearrange("p (k q) -> p k q", k=K, q=Q)
            ly3 = ly[:].rearrange("p (k q) -> p k q", k=K, q=Q)
            omx3 = omx[:].rearrange("p (k q) -> p k q", k=K, q=Q)
            lx3 = lx[:].rearrange("p (k q) -> p k q", k=K, q=Q)
            nc.vector.tensor_tensor(out=wv[:, :, 0], in0=omy3, in1=omx3,
                                    op=ALU.mult)  # y0 x0
            nc.vector.tensor_tensor(out=wv[:, :, 1], in0=omy3, in1=lx3,
                                    op=ALU.mult)  # y0 x1
            nc.vector.tensor_tensor(out=wv[:, :, 2], in0=ly3, in1=omx3,
                                    op=ALU.mult)  # y1 x0
            nc.vector.tensor_tensor(out=wv[:, :, 3], in0=ly3, in1=lx3,
                                    op=ALU.mult)  # y1 x1
            # per tap: XBAR transpose [128 Pp, 128 (l,q)] -> [128 (l,q), Pp],
            # then contiguous bounce to DRAM row (l, q, Pp) for broadcast
            for k in range(K):
                tk = tkpool.tile([P, P], bf16, tag="tk")
                nc.scalar.dma_start(out=tk[:], in_=wpre[:, k * P:(k + 1) * P],
                                    transpose=True)
                nc.scalar.dma_start(
                    out=wrow[k:k + 1, :].rearrange("o (a b) -> (o a) b", b=P),
                    in_=tk[:])

            # ------------- stage 1a: wrapped gather indices (3 chunks) ------
            # computed by DVE directly in the SWDGE wrapped-16 layout, with a
            # small rotating scratch set; per-chunk DMA loads of the wrapped
            # offsets/tables keep SBUF footprint low
            idx16 = gpool.tile([P, IDXC], i16)
            CW = IDXC // KC  # 768 cols per chunk

            def wchain_axis(offc, tabc, sfx):
                pw = gpool.tile([16, CW], f32, tag="wp" + sfx,
                                name="wp" + sfx)
                nc.vector.tensor_tensor(out=pw[:], in0=offc, in1=tabc,
                                        op=ALU.add)
                i0 = gpool.tile([16, CW], i32, tag="wi", name="wi")
                nc.vector.tensor_copy(out=i0[:], in_=pw[:])
                f0 = gpool.tile([16, CW], f32, tag="wf" + sfx,
                                name="wf" + sfx)
                nc.vector.tensor_copy(out=f0[:], in_=i0[:])
                lr = gpool.tile([16, CW], f32, tag="wl", name="wl")
                nc.vector.tensor_tensor(out=lr[:], in0=pw[:], in1=f0[:],
                                        op=ALU.subtract)
                adj = gpool.tile([16, CW], f32, tag="wa", name="wa")
                nc.vector.tensor_scalar(out=adj[:], in0=lr[:], scalar1=0.0,
                                        scalar2=None, op0=ALU.is_lt)
                fl = gpool.tile([16, CW], f32, tag="wp" + sfx,
                                name="wfl" + sfx)
                nc.vector.tensor_tensor(out=fl[:], in0=f0[:], in1=adj[:],
                                        op=ALU.subtract)
                cl = gpool.tile([16, CW], f32, tag="wf" + sfx,
                                name="wcl" + sfx)
                nc.vector.tensor_scalar(out=cl[:], in0=fl[:],
                                        scalar1=FB - PADR, scalar2=FB + 64.0,
                                        op0=ALU.max, op1=ALU.min)
                return cl

            for c in range(KC):
                cs = slice(c * CW, (c + 1) * CW)
                cs2 = slice(IDXC + c * CW, IDXC + (c + 1) * CW)
                ofy = gpool.tile([16, CW], f32, tag="ofy", name="ofy")
                nc.sync.dma_start(out=ofy[:], in_=offw_d.ap()[:, cs])
                tby = gpool.tile([16, CW], f32, tag="tby", name="tby")
                nc.sync.dma_start(out=tby[:], in_=tabw_d.ap()[:, cs])
                ofx = gpool.tile([16, CW], f32, tag="ofx", name="ofx")
                nc.sync.dma_start(out=ofx[:], in_=offw_d.ap()[:, cs2])
                tbx = gpool.tile([16, CW], f32, tag="tbx", name="tbx")
                nc.sync.dma_start(out=tbx[:], in_=tabw_d.ap()[:, cs2])
                ycw = wchain_axis(ofy[:], tby[:], "y")
                xcw = wchain_axis(ofx[:], tbx[:], "x")
                rfw = gpool.tile([16, CW], f32, tag="wl", name="rfw")
                nc.vector.scalar_tensor_tensor(
                    out=rfw[:], in0=ycw[:], scalar=float(WP), in1=xcw[:],
                    op0=ALU.mult, op1=ALU.add)
                nc.vector.tensor_scalar(
                    out=idx16[0:16, cs], in0=rfw[:],
                    scalar1=-(WP + 1.0) * FB + 2 * WP + 2.0,
                    scalar2=None, op0=ALU.add)
                # replicate to all 128 partitions (16 -> 32 -> 64 -> 128)
                nc.sync.dma_start(out=idx16[16:32, cs], in_=idx16[0:16, cs])
                nc.sync.dma_start(out=idx16[32:64, cs], in_=idx16[0:32, cs])
                nc.sync.dma_start(out=idx16[64:128, cs], in_=idx16[0:64, cs])

            # ---------------- stage 2: per-tap gather/mult/matmul -----------
            # psum quarters: (h, q) -> positions [h*2048 + q*1024, +1024)
            ps = [[None, None], [None, None]]
            for h in range(2):
                for q in range(2):
                    psq = pspool.tile([P, NQ], f32, tag=f"ps{h}{q}",
                                      name=f"ps{h}{q}")
                    ps[h][q] = psq

            for k in range(K):
                lhsT = wmat[:, k * COUT:(k + 1) * COUT]
                wbc = wpool.tile([P, 4 * NPOS], bf16, tag="wb")
                nc.scalar.dma_start(
                    out=wbc[:],
                    in_=wrow[k:k + 1, :].to_broadcast((P, 4 * NPOS)))
                wbc4 = wbc[:].rearrange("p (l i) -> p l i", l=4)
                for h in range(2):
                    m = mpool.tile([P, 4 * NH], bf16, tag="m")
                    m3 = m[:].rearrange("p (l i) -> p l i", l=4)
                    for o, n in CHUNKS:
                        g = gapool.tile([P, 4 * 896], bf16, tag="g")
                        c0 = k * 256 + h * 128 + o // 16
                        nc.gpsimd.dma_gather(
                            g[:, :4 * n].rearrange("p (j i) -> p j i", j=4),
                            xt_d.ap(),
                            idx16[:, c0:c0 + n // 16],
                            num_idxs=n, num_idxs_reg=n, elem_size=4 * P,
                            transpose=True)
                        nc.vector.tensor_tensor(
                            out=m3[:, :, o:o + n],
                            in0=g[:, :4 * n].rearrange(
                                "p (l i) -> p l i", l=4),
                            in1=wbc4[:, :, h * NH + o:h * NH + o + n],
                            op=ALU.mult)
                    for q in range(2):
                        for l in range(4):
                            for b2 in range(2):
                                c0 = b2 * 512
                                nc.tensor.matmul(
                                    ps[h][q][:, c0:c0 + 512], lhsT,
                                    m[:, l * NH + q * NQ + c0:
                                      l * NH + q * NQ + c0 + 512],
                                    start=(k == 0 and l == 0),
                                    stop=(k == K - 1 and l == 3),
                                    skip_group_check=True)

            # ---------------- stage 3: bias + store -------------------------
            for h in range(2):
                for q in range(2):
                    o = opool.tile([P, NQ], f32, tag="o")
                    nc.vector.tensor_scalar(
                        out=o[:], in0=ps[h][q][:],
                        scalar1=bias[:, 0:1], scalar2=None, op0=ALU.add)
                    p0 = h * NH + q * NQ
                    nc.scalar.dma_start(out=out_d.ap()[:, p0:p0 + NQ],
                                        in_=o[:])

    nc.compile()
    return nc


_NC_CACHE = None


def _host_inputs(x, offset, weight, bias):
    """Per-core input maps (core b <- batch b) + replicated constants."""
    wq = np.ascontiguousarray(weight, np.float32)  # [COUT, CIN, KH, KW]
    # wmat[c, k*COUT + o] = weight[o, c, ky, kx]
    wmat = wq.reshape(COUT, CIN, K).transpose(1, 2, 0).reshape(CIN, K * COUT)
    wmat = np.ascontiguousarray(wmat).astype(ml_dtypes.bfloat16)
    bias_h = np.ascontiguousarray(bias, np.float32).reshape(P, 1)
    # natural tables [Pp, axis*288 + k*Q + q] (pos p = q*128 + Pp)
    p_of = (np.arange(Q)[:, None] * P + np.arange(P)[None, :])  # [Q, P]
    ho = (p_of // WO).astype(np.float32)
    wo = (p_of % WO).astype(np.float32)
    tab2 = np.empty((P, 2 * K * Q), np.float32)
    for k in range(K):
        tab2[:, k * Q:(k + 1) * Q] = (ho + (k // 3 - 1) + FB).T
        tab2[:, K * Q + k * Q:K * Q + (k + 1) * Q] = (wo + (k % 3 - 1) + FB).T
    # wrapped tables [r, axis*2304 + k*256 + q*8 + g] (pos p = q*128+g*16+r)
    rr = np.arange(16)[:, None, None, None]
    kk = np.arange(K)[None, :, None, None]
    qq = np.arange(Q)[None, None, :, None]
    gg = np.arange(8)[None, None, None, :]
    pw = qq * P + gg * 16 + rr                      # [16, K, Q, 8]
    how = (pw // WO).astype(np.float32)
    wow = (pw % WO).astype(np.float32)
    tyw = how + (kk // 3 - 1) + FB
    txw = wow + (kk % 3 - 1) + FB
    tabw = np.concatenate(
        [tyw.reshape(16, IDXC), txw.reshape(16, IDXC)], axis=1
    ).astype(np.float32)
    in_maps = []
    for b in range(B):
        ob = np.ascontiguousarray(offset[b], np.float32).reshape(2 * K, NPOS)
        # natural off2[Pp, axis*288 + k*Q + q] = offset[2k+axis, q*128+Pp]
        off2 = np.empty((P, 2 * K * Q), np.float32)
        oy = ob[0::2].reshape(K, Q, P)  # [k, q, Pp]
        ox = ob[1::2].reshape(K, Q, P)
        off2[:, :K * Q] = oy.transpose(2, 0, 1).reshape(P, K * Q)
        off2[:, K * Q:] = ox.transpose(2, 0, 1).reshape(P, K * Q)
        # wrapped offw[r, axis*2304 + k*256 + q*8 + g]
        oyw = oy.reshape(K, Q, 8, 16)   # [k, q, g, r]
        oxw = ox.reshape(K, Q, 8, 16)
        offw = np.concatenate(
            [oyw.transpose(3, 0, 1, 2).reshape(16, IDXC),
             oxw.transpose(3, 0, 1, 2).reshape(16, IDXC)], axis=1
        ).astype(np.float32)
        # xt4: padded position-major bf16, 4 corner rows packed per entry
        xp = np.zeros((HP, WP, P), np.float32)
        xp[PADR:PADR + H, PADR:PADR + W, :] = (
            np.ascontiguousarray(x[b], np.float32).transpose(1, 2, 0))
        flat = np.zeros((NE + WP + 2, P), np.float32)
        flat[:NE] = xp.reshape(NE, P)
        xt = np.empty((NE, 4 * P), np.float32)
        xt[:, 0 * P:1 * P] = flat[0:NE]
        xt[:, 1 * P:2 * P] = flat[1:NE + 1]
        xt[:, 2 * P:3 * P] = flat[WP:NE + WP]
        xt[:, 3 * P:4 * P] = flat[WP + 1:NE + WP + 1]
        in_maps.append({
            "xt": xt.astype(ml_dtypes.bfloat16),
            "offw": offw,
            "tabw": tabw,
            "off2": off2,
            "tab2": tab2,
            "wmat": wmat,
            "bias": bias_h,
        })
    return in_maps


def kernel(x, offset, weight, bias):
    global _NC_CACHE
    from concourse.bass_utils import run_bass_kernel_spmd

    if _NC_CACHE is None:
        _NC_CACHE = _build_kernel()
    nc = _NC_CACHE
    in_maps = _host_inputs(x, offset, weight, bias)
    res = run_bass_kernel_spmd(nc, in_maps, list(range(B)))
    out = np.stack([res.results[b]["out"].reshape(COUT, HO, WO) for b in range(B)])
    return out.astype(np.float32)


if __name__ == "__main__":
    import sys
    d = np.load("/tmp/inputs.npz")
    if len(sys.argv) > 1 and sys.argv[1] == "sim":
        from concourse.bass_interp import CoreSim
        nc = _build_kernel()
        in_maps = _host_inputs(d["x"], d["offset"], d["weight"], d["bias"])
        sim = CoreSim(nc)
        for kk, vv in in_maps[0].items():
            sim.tensor(kk)[:] = vv
        sim.simulate()
        out = np.asarray(sim.tensor("out")).reshape(1, COUT, HO, WO)
        exp = np.load("/tmp/expected.npy")[0:1]
    else:
        out = kernel(d["x"], d["offset"], d["weight"], d["bias"])
        exp = np.load("/tmp/expected.npy")
    err = np.abs(out - exp)
    print("rel l2:", np.linalg.norm(out - exp) / np.linalg.norm(exp))
    print("absmax rel:", err.max() / np.abs(exp).max())
